# revision 41
# baseline (speedup 1.0000x reference)
"""BiLSTM-CRF loss kernel for 8 Trainium2 NeuronCores.

Sharding: batch-only. Core c owns batches [8c, 8c+8) and runs BOTH LSTM
directions for them as two independent recurrence streams (A=forward,
B=backward on host-time-reversed input), interleaved so that one stream's
sigmoid/tanh/mul chain executes while the other stream's 64 W_hh matmuls
occupy the PE. This removes the per-step PE idle bubble of a single-stream
schedule and needs no inter-core collective: each core locally owns full
emissions for its 8 batches and runs the CRF (exp-space recurrence in bf16
with power-of-2 renormalization) plus the gold-path score (host-precomputed
one-hot matmuls).

Self-contained: hardcodes all shapes; no sibling imports.
"""

import numpy as np
import ml_dtypes

import concourse.bass as bass
import concourse.tile as tile
from concourse import mybir
from concourse.bass_utils import run_bass_kernel_spmd

F32 = mybir.dt.float32
BF16 = mybir.dt.bfloat16
I32 = mybir.dt.int32
AF = mybir.ActivationFunctionType
ALU = mybir.AluOpType

N_CORES = 8
B, T, E, H, K = 64, 256, 256, 512, 32
START, END = 30, 31
BC = 8    # batch per core
LN2 = float(np.log(2.0))


# ---------------------------------------------------------------------------
# walrus-compat: this container's walrus supports only ONE sync-wait per
# instruction; Tile sometimes emits more. Split extras onto same-engine NOPs
# inserted just before the offending instruction.
# ---------------------------------------------------------------------------
def _split_multiwait(nc):
    import bass_rust
    n = 0
    for f in nc.m.functions:
        for bb in f.blocks:
            insts = bb.instructions
            if not insts:
                continue
            out = []
            changed = False
            for ins in insts:
                si = ins.sync_info
                if si is not None and si.on_wait and len(si.on_wait) > 1:
                    waits = list(si.on_wait)
                    eng = nc.engines[ins.engine]
                    for w in waits[:-1]:
                        nop = eng.nop()
                        nop_ins = nop.ins
                        cur_list = nc.cur_bb.bb.instructions
                        assert cur_list and cur_list[-1].name == nop_ins.name
                        cur_list.pop()
                        nop_ins.sync_info = bass_rust.SyncInfo(
                            on_wait=[w], on_update=[]
                        )
                        out.append(nop_ins)
                        n += 1
                    si.on_wait = [waits[-1]]
                    ins.sync_info = si
                    changed = True
                out.append(ins)
            if changed:
                bb.instructions = out
    return n


# ---------------------------------------------------------------------------
# Strip per-matmul completion increments. Every MATMUL increments the PE
# semaphore at completion and these EVT_SEM writes serialize (~26ns each), so
# with 130+ matmuls per step-pair the completion counter lags issue by a full
# pair, stalling everything that waits on "group complete". Keep only the
# increments whose cumulative value some wait actually targets and remap all
# thresholds.
# ---------------------------------------------------------------------------
def _strip_mm_incs(nc):
    blocks = [bb for f in nc.m.functions for bb in f.blocks]
    # semaphores incremented by matmuls
    mm_sems = set()
    for bb in blocks:
        for ins in bb.instructions:
            si = ins.sync_info
            if si is None or not si.on_update:
                continue
            if type(ins).__name__ == 'InstMatmult':
                for u in si.on_update:
                    if u.update_mode == 'sem-inc':
                        mm_sems.add(u.id)
    stripped = 0
    for sem in mm_sems:
        # thresholds awaited on this semaphore
        targets = set()
        for bb in blocks:
            for ins in bb.instructions:
                si = ins.sync_info
                if si is None:
                    continue
                for w in (si.on_wait or []):
                    if w.id == sem and w.wait_mode == 'sem-ge-imm':
                        targets.add(w.wait_value)
        # walk updates in order; keep matmul incs only at awaited counts
        cum = 0
        keep_cum = []          # sorted kept cumulative values
        for bb in blocks:
            for ins in bb.instructions:
                si = ins.sync_info
                if si is None or not si.on_update:
                    continue
                ups = list(si.on_update)
                new_ups = []
                for u in ups:
                    if u.id != sem or u.update_mode != 'sem-inc':
                        new_ups.append(u)
                        continue
                    assert u.update_value == 1
                    cum += 1
                    if type(ins).__name__ == 'InstMatmult' and \
                            cum not in targets:
                        stripped += 1
                    else:
                        keep_cum.append(cum)
                        new_ups.append(u)
                if len(new_ups) != len(ups):
                    si.on_update = new_ups
                    ins.sync_info = si
        import bisect
        for bb in blocks:
            for ins in bb.instructions:
                si = ins.sync_info
                if si is None or not si.on_wait:
                    continue
                ch = False
                ws = list(si.on_wait)
                for w in ws:
                    if w.id == sem and w.wait_mode == 'sem-ge-imm':
                        nv = bisect.bisect_right(keep_cum, w.wait_value)
                        if nv != w.wait_value:
                            w.wait_value = nv
                            ch = True
                if ch:
                    si.on_wait = ws
                    ins.sync_info = si
    return stripped


# ---------------------------------------------------------------------------
# device program
# ---------------------------------------------------------------------------
def build_nc(t_steps=T, n_cores=N_CORES):
    TS = t_steps
    TB = BC * TS           # (t, b) columns per stream
    BT = BC * TS           # (b, t) columns for emissions
    N_EV = (TS - 1) // 8   # renorm events

    nc = bass.Bass("TRN2", target_bir_lowering=False, debug=False,
                   num_devices=n_cores)

    # inputs (all staged per-core on host)
    xT = nc.dram_tensor("xT", [4, 128, TB], BF16, kind="ExternalInput")
    wihT = nc.dram_tensor("wihT", [4, 128, 4 * H], BF16, kind="ExternalInput")
    whhT = nc.dram_tensor("whhT", [8, 128, 4 * H], BF16, kind="ExternalInput")
    biasT = nc.dram_tensor("biasT", [128, 32], F32, kind="ExternalInput")
    woutT = nc.dram_tensor("woutT", [8, 128, K], BF16, kind="ExternalInput")
    bout = nc.dram_tensor("bout", [K, 1], F32, kind="ExternalInput")
    trans = nc.dram_tensor("trans", [K, K], F32, kind="ExternalInput")
    trans8 = nc.dram_tensor("trans8", [K, BC * K], F32, kind="ExternalInput")
    transS = nc.dram_tensor("transS", [K, 1], F32, kind="ExternalInput")
    transE = nc.dram_tensor("transE", [K, 1], F32, kind="ExternalInput")
    tagsOH = nc.dram_tensor("tagsOH", [128, BC * 3 * 2 * K], BF16,
                            kind="ExternalInput")
    emM = nc.dram_tensor("emM", [K, BT], F32, kind="ExternalInput")
    ident = nc.dram_tensor("ident", [128, 128], BF16, kind="ExternalInput")
    out = nc.dram_tensor("out", [1, BC], F32, kind="ExternalOutput")

    with tile.TileContext(nc) as tc:
        _body(tc, locals(), TS, TB, BT, N_EV)
    _strip_mm_incs(nc)
    return nc


def _body(tc, io, TS, TB, BT, N_EV):
    from contextlib import ExitStack
    nc = tc.nc
    xT, wihT, whhT, biasT, woutT = io['xT'], io['wihT'], io['whhT'], io['biasT'], io['woutT']
    bout, trans, trans8 = io['bout'], io['trans'], io['trans8']
    transS, transE = io['transS'], io['transE']
    tagsOH, emM, ident, out = io['tagsOH'], io['emM'], io['ident'], io['out']

    WIN = min(32, TS)
    NW = TS // WIN
    assert TS % WIN == 0

    with ExitStack() as top:
        persist = top.enter_context(tc.tile_pool(name="persist", bufs=1))

        # persistent tiles
        bias_sb = persist.tile([128, 32], F32)
        nc.sync.dma_start(bias_sb[:], biasT[:, :])
        ident_sb = persist.tile([128, 128], BF16)
        nc.sync.dma_start(ident_sb[:], ident[:, :])
        trans_sb = persist.tile([K, K], F32)
        nc.sync.dma_start(trans_sb[:], trans[:, :])
        trans8_sb = persist.tile([K, BC * K], F32)
        nc.sync.dma_start(trans8_sb[:], trans8[:, :])
        transS_sb = persist.tile([K, 1], F32)
        nc.sync.dma_start(transS_sb[:], transS[:, :])
        transE_sb = persist.tile([K, 1], F32)
        nc.sync.dma_start(transE_sb[:], transE[:, :])
        tagsOH_sb = persist.tile([128, BC * 3 * 2 * K], BF16)
        nc.sync.dma_start(tagsOH_sb[:], tagsOH[:, :])
        emM_sb = persist.tile([K, BT], F32)
        nc.sync.dma_start(emM_sb[:], emM[:, :])
        bout_sb = persist.tile([K, 1], F32)
        nc.sync.dma_start(bout_sb[:], bout[:, :])
        ones32_bf = persist.tile([K, 1], BF16)
        nc.vector.memset(ones32_bf[:], 1.0)
        ones32_f = persist.tile([K, 1], F32)
        nc.vector.memset(ones32_f[:], 1.0)
        ones1x32 = persist.tile([1, K], F32)
        nc.vector.memset(ones1x32[:], 1.0)

        # per-stream weights / inputs
        wih_sb, whh_sb, wout_sb, x_sb, xg_sb, em_sb = [], [], [], [], [], []
        for s in range(2):
            wih = persist.tile([128, 2 * 4 * H], BF16, tag=f"wih{s}")
            for e in range(2):
                nc.sync.dma_start(wih[:, e * 4 * H:(e + 1) * 4 * H],
                                  wihT[2 * s + e, :, :])
            wih_sb.append(wih)
            whh = persist.tile([128, 4 * 4 * H], BF16, tag=f"whh{s}")
            for c in range(4):
                nc.sync.dma_start(whh[:, c * 4 * H:(c + 1) * 4 * H],
                                  whhT[4 * s + c, :, :])
            whh_sb.append(whh)
            wo = persist.tile([128, 4 * K], BF16, tag=f"wo{s}")
            for c in range(4):
                nc.sync.dma_start(wo[:, c * K:(c + 1) * K],
                                  woutT[4 * s + c, :, :])
            wout_sb.append(wo)
            xs = persist.tile([128, 2 * TB], BF16, tag=f"xs{s}")
            for e in range(2):
                nc.sync.dma_start(xs[:, e * TB:(e + 1) * TB],
                                  xT[2 * s + e, :, :])
            x_sb.append(xs)
            # xg ring: 2 windows x [j, t_in_win, b], separate tiles per
            # window parity so staging writes can't falsely order against
            # the recurrence's reads of the other window
            xga = persist.tile([128, 16 * WIN * BC], BF16, tag=f"xga{s}")
            xgb = persist.tile([128, 16 * WIN * BC], BF16, tag=f"xgb{s}")
            xg_sb.append((xga, xgb))
            em = persist.tile([K, BT], F32, tag=f"em{s}")
            em_sb.append(em)
        # full hidden-state history [t, c_chunk, b] per stream; written once
        # per step by the chain, read by the next burst and the bulk emission
        # pass (avoids per-step emission matmuls + PSUM pressure in the loop)
        h_hist = []
        for s in range(2):
            hht = persist.tile([128, TS * 4 * BC], BF16, tag=f"hh{s}",
                               name=f"hh{s}")
            h_hist.append(hht)

        # small tiles that cross phase boundaries
        e_tot = persist.tile([1, BC], F32)
        gold = persist.tile([1, BC], F32)
        expE = persist.tile([K, BT], F32)
        em_fin = persist.tile([K, BT], F32)

        # ---------------- main loop: 2-stream LSTM ---------------------------
        with ExitStack() as c_stack:
            xgps = c_stack.enter_context(
                tc.tile_pool(name="xgps", bufs=2, space="PSUM"))
            gpsum = c_stack.enter_context(
                tc.tile_pool(name="gpsum", bufs=1, space="PSUM"))
            spool = c_stack.enter_context(tc.tile_pool(name="spool", bufs=1))
            qpool = c_stack.enter_context(tc.tile_pool(name="qpool", bufs=3))

            def stage_xg(s, w, j):
                # compute Xg for stream s, window w, gate-chunk j into the ring
                ps = xgps.tile([128, WIN * BC], F32, tag=f"xgps{j % 2}",
                               name="ps", bufs=1)
                for e in range(2):
                    nc.tensor.matmul(
                        ps[:],
                        wih_sb[s][:, e * 4 * H + j * 128:
                                  e * 4 * H + (j + 1) * 128],
                        x_sb[s][:, e * TB + w * WIN * BC:
                                e * TB + (w + 1) * WIN * BC],
                        start=(e == 0), stop=(e == 1))
                slot = j * WIN * BC
                nc.vector.tensor_scalar_add(
                    xg_sb[s][w % 2][:, slot:slot + WIN * BC], ps[:],
                    bias_sb[:, s * 16 + j:s * 16 + j + 1])

            # prologue: stage window 0 for both streams; zero h/c
            h0 = spool.tile([128, 4 * BC], BF16, tag="h0")
            nc.vector.memset(h0[:], 0.0)
            c_cur = []
            for s in range(2):
                for j in range(16):
                    stage_xg(s, 0, j)
                c = spool.tile([128, 4 * BC], F32, tag=f"c{s}")
                nc.vector.memset(c[:], 0.0)
                c_cur.append(c)
            hh_v = [h_hist[s][:].rearrange("p (t cb) -> p t cb", t=TS)
                    for s in range(2)]
            h_cur = [h0[:], h0[:]]

            for t in range(TS):
                w, tw = t // WIN, t % WIN
                for s in range(2):
                    # PE burst, two half acc-groups in separate banks.
                    # Gate row order is g,i,f,o: half 0 = (g,i) finishes at
                    # half-burst so tanh(g), sig(i), i*g overlap half 1's MMs.
                    xg_v = xg_sb[s][w % 2][:].rearrange(
                        "p (j t b) -> p j t b", j=16, t=WIN)
                    gh = []
                    for grp in range(2):
                        gps = gpsum.tile([128, 8 * BC], F32,
                                         tag=f"g{s}{grp}_{t % 2}" if grp else f"g{s}0",
                                         name="gps", bufs=1)
                        nc.tensor.matmul(
                            gps[:], ident_sb[:],
                            xg_v[:, 8 * grp:8 * grp + 8, tw, :],
                            start=True, stop=False)
                        for c_in in range(4):
                            for j in range(8 * grp, 8 * grp + 8):
                                nc.tensor.matmul(
                                    gps[:, (j - 8 * grp) * BC:
                                        (j - 8 * grp + 1) * BC],
                                    whh_sb[s][:, c_in * 4 * H + j * 128:
                                              c_in * 4 * H + (j + 1) * 128],
                                    h_cur[s][:, c_in * BC:(c_in + 1) * BC],
                                    start=False,
                                    stop=(c_in == 3 and j == 8 * grp + 7))
                        gh.append(gps)
                        if grp == 0:
                            # chain part 1 runs under half-group 1's matmuls
                            tgg = qpool.tile([128, 4 * BC], F32,
                                             tag=f"tgg{s}{t % 2}")
                            nc.scalar.activation(tgg[:], gh[0][:, 0:4 * BC],
                                                 AF.Tanh)
                            sgi = qpool.tile([128, 4 * BC], F32,
                                             tag=f"sgi{s}{t % 2}")
                            nc.scalar.activation(sgi[:],
                                                 gh[0][:, 4 * BC:8 * BC],
                                                 AF.Sigmoid)
                            tmp = qpool.tile([128, 4 * BC], F32,
                                             tag=f"tmp{s}{t % 2}")
                            nc.vector.tensor_mul(tmp[:], sgi[:], tgg[:])
                    # chain part 2 after half-group 1 (f,o)
                    sfo = qpool.tile([128, 8 * BC], F32, tag=f"sfo{s}{t % 2}")
                    nc.scalar.activation(sfo[:], gh[1][:], AF.Sigmoid)
                    cn = qpool.tile([128, 4 * BC], F32, tag=f"cn{s}{t % 2}")
                    nc.vector.tensor_mul(cn[:], sfo[:, 0:4 * BC], c_cur[s][:])
                    nc.vector.tensor_add(cn[:], cn[:], tmp[:])
                    tc_sb = qpool.tile([128, 4 * BC], F32, tag=f"tc{s}{t % 2}")
                    nc.scalar.activation(tc_sb[:], cn[:], AF.Tanh)
                    hn = hh_v[s][:, t, :]
                    nc.vector.tensor_mul(hn, sfo[:, 4 * BC:8 * BC], tc_sb[:])
                    h_cur[s], c_cur[s] = hn, cn
                # stage next window's Xg, spread over this window's steps
                if w + 1 < NW and tw < 32:
                    stage_xg(tw % 2, w + 1, tw // 2)

        # bulk emission pass: em[k, (t,b)] = W_out_dir^T h_hist
        with ExitStack() as em_stack:
            embps = em_stack.enter_context(
                tc.tile_pool(name="embps", bufs=2, space="PSUM"))
            NCH = min(512, TS * BC)
            for s in range(2):
                hh4 = h_hist[s][:].rearrange(
                    "p (t c b) -> p t c b", t=TS, c=4)
                em_tv = em_sb[s][:].rearrange("p (b t) -> p t b", b=BC)
                for kch in range(TS * BC // NCH):
                    tlo = kch * (NCH // BC)
                    thi = (kch + 1) * (NCH // BC)
                    ps = embps.tile([K, NCH], F32, tag="emb")
                    for c in range(4):
                        nc.tensor.matmul(
                            ps[:], wout_sb[s][:, c * K:(c + 1) * K],
                            hh4[:, tlo:thi, c, :],
                            start=(c == 0), stop=(c == 3))
                    nc.vector.tensor_copy(
                        em_tv[:, tlo:thi, :],
                        ps[:].rearrange("p (t b) -> p t b", b=BC))

        # ---------------- finalize emissions --------------------------------
        # em_fin[k, (b,t)] = em_fwd + bias + em_bwd(reversed stream steps)
        em_f_v = em_sb[0][:].rearrange("p (b t) -> p b t", b=BC)
        em_b_rv = em_sb[1][:].rearrange("p (b t) -> p b t", b=BC)[:, :, ::-1]
        em_fin_v = em_fin[:].rearrange("p (b t) -> p b t", b=BC)
        nc.vector.scalar_tensor_tensor(
            em_fin_v, em_f_v, bout_sb[:, 0:1], em_b_rv, ALU.add, ALU.add)
        nc.scalar.activation(expE[:], em_fin[:], AF.Exp)

        # ---------------- gold-path score -----------------------------------
        with ExitStack() as e_stack:
            epool = e_stack.enter_context(tc.tile_pool(name="epool", bufs=2))
            epsum = e_stack.enter_context(
                tc.tile_pool(name="epsum", bufs=2, space="PSUM"))

            # transition counts via host-precomputed pair one-hots
            oh_v = tagsOH_sb[:].rearrange(
                "p (b c n k) -> p b c n k", b=BC, c=3, n=2)
            C_ps = epsum.tile([K, BC * K], F32, tag="cps")
            for b in range(BC):
                for ch in range(3):
                    nc.tensor.matmul(C_ps[:, b * K:(b + 1) * K],
                                     oh_v[:, b, ch, 0, :],
                                     oh_v[:, b, ch, 1, :],
                                     start=(ch == 0), stop=(ch == 2))
            tcmul = epool.tile([K, BC * K], F32, tag="tcmul")
            nc.vector.tensor_mul(tcmul[:], C_ps[:], trans8_sb[:])
            tred = epool.tile([K, BC], F32, tag="tred")
            nc.vector.tensor_reduce(
                tred[:], tcmul[:].rearrange("p (b k) -> p b k", b=BC),
                mybir.AxisListType.X, ALU.add)

            # emission scores: host mask * emissions, reduce over t
            emul = epool.tile([K, BT], F32, tag="emul")
            nc.vector.tensor_mul(emul[:], emM_sb[:], em_fin[:])
            ered = epool.tile([K, BC], F32, tag="ered")
            nc.vector.tensor_reduce(
                ered[:], emul[:].rearrange("p (b t) -> p b t", b=BC),
                mybir.AxisListType.X, ALU.add)
            nc.vector.tensor_add(tred[:], tred[:], ered[:])
            g_ps = epsum.tile([1, BC], F32, tag="gps")
            nc.tensor.matmul(g_ps[:], ones32_f[:], tred[:],
                             start=True, stop=True)
            nc.vector.tensor_copy(gold[:], g_ps[:])

        # ---------------- CRF forward recurrence ----------------------------
        with ExitStack() as f_stack:
            fpool = f_stack.enter_context(tc.tile_pool(name="fpool", bufs=2))
            fpsum = f_stack.enter_context(
                tc.tile_pool(name="fpsum", bufs=2, space="PSUM"))

            expT_sb = fpool.tile([K, K], BF16, tag="expT")
            nc.scalar.activation(expT_sb[:], trans_sb[:], AF.Exp)
            expTs = fpool.tile([K, 1], F32, tag="expTs")
            nc.scalar.activation(expTs[:], transS_sb[:], AF.Exp)
            expTe = fpool.tile([K, 1], BF16, tag="expTe")
            nc.scalar.activation(expTe[:], transE_sb[:], AF.Exp)

            expE_v = expE[:].rearrange("p (b t) -> p b t", b=BC)
            k_acc = fpool.tile([1, BC], I32, tag="kacc")
            nc.vector.memset(k_acc[:], 0)
            a_cur = fpool.tile([K, BC], BF16, tag="a")
            nc.vector.tensor_scalar_mul(a_cur[:], expE_v[:, :, 0], expTs[:])

            for t in range(1, TS):
                a_ps = fpsum.tile([K, BC], F32, tag="aps")
                nc.tensor.matmul(a_ps[:], expT_sb[:], a_cur[:],
                                 start=True, stop=True)
                a_nxt = fpool.tile([K, BC], BF16, tag="a")
                nc.vector.tensor_mul(a_nxt[:], a_ps[:], expE_v[:, :, t])
                a_cur = a_nxt
                if t % 8 == 0:
                    zps_t = fpsum.tile([K, BC], F32, tag="fps")
                    z_ps = zps_t[0:1, :]
                    nc.tensor.matmul(z_ps[:], ones32_bf[:], a_cur[:],
                                     start=True, stop=True)
                    z_sb = fpool.tile([1, BC], F32, tag="zsb")
                    nc.vector.tensor_copy(z_sb[:], z_ps[:])
                    e_i = fpool.tile([1, BC], I32, tag="ei")
                    nc.vector.tensor_scalar(e_i[:], z_sb[:].bitcast(I32),
                                            23, None,
                                            ALU.logical_shift_right)
                    nc.vector.tensor_add(k_acc[:], k_acc[:], e_i[:])
                    sc_i = fpool.tile([1, BC], I32, tag="sci")
                    nc.vector.tensor_scalar(sc_i[:], e_i[:], -1, 254,
                                            ALU.mult, ALU.add)
                    nc.vector.tensor_scalar(sc_i[:], sc_i[:], 23, None,
                                            ALU.logical_shift_left)
                    bc_ps = fpsum.tile([K, BC], F32, tag="fps")
                    nc.tensor.matmul(bc_ps[:], ones1x32[:],
                                     sc_i[:].bitcast(F32),
                                     start=True, stop=True)
                    a_sc = fpool.tile([K, BC], BF16, tag="a")
                    nc.vector.tensor_mul(a_sc[:], a_cur[:], bc_ps[:])
                    a_cur = a_sc

            zf_t = fpsum.tile([K, BC], F32, tag="fps")
            zf_ps = zf_t[0:1, :]
            nc.tensor.matmul(zf_ps[:], expTe[:], a_cur[:],
                             start=True, stop=True)
            logz = fpool.tile([1, BC], F32, tag="logz")
            nc.scalar.activation(logz[:], zf_ps[:], AF.Ln)
            k_f = fpool.tile([1, BC], F32, tag="kf")
            nc.vector.tensor_copy(k_f[:], k_acc[:])
            # nll = logz + ln2*(sum e) - 127*ln2*n_ev - gold
            nll = fpool.tile([1, BC], F32, tag="nll")
            nc.vector.tensor_scalar(nll[:], k_f[:], LN2,
                                    -127.0 * LN2 * N_EV, ALU.mult, ALU.add)
            nc.vector.tensor_add(nll[:], nll[:], logz[:])
            nc.vector.tensor_sub(nll[:], nll[:], gold[:])
            nc.sync.dma_start(out[:, :], nll[:])


# ---------------------------------------------------------------------------
# host side
# ---------------------------------------------------------------------------
def _perm_rows(W):
    # gate-major blocks reordered g,i,f,o (pytorch order is i,f,g,o)
    out = np.empty_like(W)
    out[0:512] = W[1024:1536]        # g
    out[512:1024] = W[0:512]         # i
    out[1024:1536] = W[512:1024]     # f
    out[1536:2048] = W[1536:2048]    # o
    return out


def make_in_maps(inputs, t_steps=T):
    TS = t_steps
    X = np.asarray(inputs['X'], np.float32)
    tags = np.asarray(inputs['tags']).astype(np.int64)
    W = {d: (np.asarray(inputs[f'W_ih_{d}'], np.float32),
             np.asarray(inputs[f'W_hh_{d}'], np.float32),
             np.asarray(inputs[f'b_ih_{d}'], np.float32)
             + np.asarray(inputs[f'b_hh_{d}'], np.float32))
         for d in ('f', 'b')}
    W_out = np.asarray(inputs['W_out'], np.float32)
    b_out = np.asarray(inputs['b_out'], np.float32)
    trans = np.asarray(inputs['transitions'], np.float32)

    # per-direction static tensors
    wih_all, whh_all, wout_all, bias_all = [], [], [], []
    for d in ('f', 'b'):
        Wih, Whh, bsum = W[d]
        wih_all.append(_perm_rows(Wih).T.astype(ml_dtypes.bfloat16)
                       .reshape(2, 128, 4 * H))
        whh_all.append(_perm_rows(Whh).T.astype(ml_dtypes.bfloat16)
                       .reshape(4, 128, 4 * H))
        wo = W_out[(0 if d == 'f' else H):(H if d == 'f' else 2 * H), :]
        wout_all.append(wo.reshape(4, 128, K).astype(ml_dtypes.bfloat16))
        bias_all.append(_perm_rows(bsum[:, None])[:, 0]
                        .reshape(16, 128).T.copy())
    wihT = np.ascontiguousarray(np.concatenate(wih_all, 0))
    whhT = np.ascontiguousarray(np.concatenate(whh_all, 0))
    woutT = np.ascontiguousarray(np.concatenate(wout_all, 0))
    biasT = np.ascontiguousarray(np.concatenate(bias_all, 1)
                                 .astype(np.float32))        # [128, 32]

    maps = []
    for c in range(N_CORES):
        b0 = BC * c
        Xs = X[b0:b0 + BC, :TS, :]                           # [8, TS, E]
        XTf = Xs.transpose(2, 1, 0)                          # [E, TS, 8]
        XTb = XTf[:, ::-1, :]
        xT = np.concatenate(
            [XTf.reshape(2, 128, TS * BC), XTb.reshape(2, 128, TS * BC)],
            0).astype(ml_dtypes.bfloat16)

        ctags = tags[b0:b0 + BC, :TS]
        # pair one-hots over extended sequence (START, tags..., END)
        ext = np.concatenate(
            [np.full((BC, 1), START), ctags, np.full((BC, 1), END)],
            1)                                               # [8, TS+2]
        npair = TS + 1
        oh = np.zeros((128, BC, 3, 2, K), np.float32)
        for b in range(BC):
            prev, nxt = ext[b, :-1], ext[b, 1:]
            for ch in range(3):
                lo = ch * 128
                sz = min(128, max(0, npair - lo))
                r = np.arange(sz)
                oh[r, b, ch, 0, prev[lo:lo + sz]] = 1.0
                oh[r, b, ch, 1, nxt[lo:lo + sz]] = 1.0
        tagsOH = oh.reshape(128, -1).astype(ml_dtypes.bfloat16)

        emM = np.zeros((K, BC * TS), np.float32)
        for b in range(BC):
            emM[ctags[b], b * TS + np.arange(TS)] = 1.0

        maps.append({
            "xT": np.ascontiguousarray(xT),
            "wihT": wihT,
            "whhT": whhT,
            "biasT": biasT,
            "woutT": woutT,
            "bout": b_out[:, None].astype(np.float32),
            "trans": trans,
            "trans8": np.ascontiguousarray(np.tile(trans, (1, BC))),
            "transS": np.ascontiguousarray(trans[START, :][:, None]),
            "transE": np.ascontiguousarray(trans[:, END][:, None]),
            "tagsOH": np.ascontiguousarray(tagsOH),
            "emM": emM,
            "ident": np.eye(128, dtype=ml_dtypes.bfloat16),
        })
    return maps


def assemble_out(results):
    nll = np.zeros(B, np.float32)
    for c in range(N_CORES):
        nll[BC * c:BC * (c + 1)] = results[c]["out"][0]
    return nll


_CACHED = {}


def kernel(**inputs):
    masks = np.asarray(inputs['masks'], np.float32)
    assert np.all(masks == 1.0), "kernel assumes masks == 1 (setup_inputs)"
    if 'nc' not in _CACHED:
        nc = build_nc()
        _split_multiwait(nc)
        _CACHED['nc'] = nc
    in_maps = make_in_maps(inputs)
    res = run_bass_kernel_spmd(_CACHED['nc'], in_maps,
                               core_ids=list(range(N_CORES)))
    return assemble_out(res.results)


# revision 42
# speedup vs baseline: 1.1281x; 1.1281x over previous
"""BiLSTM-CRF loss kernel for 8 Trainium2 NeuronCores.

Sharding: direction x batch. Even cores run the forward LSTM, odd cores the
backward LSTM (on host-time-reversed input). Core pair (2w, 2w+1) owns batch
window [16w, 16w+16). Each core computes its direction's partial emissions
(W_out matmul fused into the recurrence), the pair exchanges partials with one
ReduceScatter, and each core then runs the CRF (factored exp-space recurrence:
one 32x32 matmul + one elementwise multiply per step, with power-of-2
renormalization every 8 steps) plus the gold-path score (one-hot / transition
count-matrix matmuls) for 8 batches, producing nll[8].

Self-contained: hardcodes all shapes; no sibling imports.
"""

import numpy as np
import ml_dtypes

import concourse.bass as bass
import concourse.tile as tile
from concourse import mybir
from concourse.tile import add_dep_helper
from concourse.bass_utils import run_bass_kernel_spmd

F32 = mybir.dt.float32
BF16 = mybir.dt.bfloat16
I32 = mybir.dt.int32
AF = mybir.ActivationFunctionType
ALU = mybir.AluOpType

N_CORES = 8
B, T, E, H, K = 64, 256, 256, 512, 32
START, END = 30, 31
BL = 16   # batch per LSTM core
BC = 8    # batch per CRF core
LN2 = float(np.log(2.0))


# ---------------------------------------------------------------------------
# walrus-compat: this container's walrus supports only ONE sync-wait per
# instruction; Tile sometimes emits more. Split extras onto same-engine NOPs
# inserted just before the offending instruction.
# ---------------------------------------------------------------------------
def _split_multiwait(nc):
    import bass_rust
    n = 0
    for f in nc.m.functions:
        for bb in f.blocks:
            insts = bb.instructions
            if not insts:
                continue
            out = []
            changed = False
            for ins in insts:
                si = ins.sync_info
                if si is not None and si.on_wait and len(si.on_wait) > 1:
                    waits = list(si.on_wait)
                    eng = nc.engines[ins.engine]
                    for w in waits[:-1]:
                        nop = eng.nop()
                        nop_ins = nop.ins
                        cur_list = nc.cur_bb.bb.instructions
                        assert cur_list and cur_list[-1].name == nop_ins.name
                        cur_list.pop()
                        nop_ins.sync_info = bass_rust.SyncInfo(
                            on_wait=[w], on_update=[]
                        )
                        out.append(nop_ins)
                        n += 1
                    si.on_wait = [waits[-1]]
                    ins.sync_info = si
                    changed = True
                out.append(ins)
            if changed:
                bb.instructions = out
    return n


# ---------------------------------------------------------------------------
# Strip per-matmul completion increments. Every MATMUL increments the PE
# semaphore at completion and these EVT_SEM writes serialize (~26ns each), so
# the completion counter lags issue and everything waiting on "group
# complete" stalls. Keep only the increments whose cumulative value some wait
# actually targets and remap all thresholds.
# ---------------------------------------------------------------------------
def _strip_mm_incs(nc):
    blocks = [bb for f in nc.m.functions for bb in f.blocks]
    mm_sems = set()
    for bb in blocks:
        for ins in bb.instructions:
            si = ins.sync_info
            if si is None or not si.on_update:
                continue
            if type(ins).__name__ == 'InstMatmult':
                for u in si.on_update:
                    if u.update_mode == 'sem-inc':
                        mm_sems.add(u.id)
    stripped = 0
    for sem in mm_sems:
        targets = set()
        for bb in blocks:
            for ins in bb.instructions:
                si = ins.sync_info
                if si is None:
                    continue
                for w in (si.on_wait or []):
                    if w.id == sem and w.wait_mode == 'sem-ge-imm':
                        targets.add(w.wait_value)
        cum = 0
        keep_cum = []
        for bb in blocks:
            for ins in bb.instructions:
                si = ins.sync_info
                if si is None or not si.on_update:
                    continue
                ups = list(si.on_update)
                new_ups = []
                for u in ups:
                    if u.id != sem or u.update_mode != 'sem-inc':
                        new_ups.append(u)
                        continue
                    assert u.update_value == 1
                    cum += 1
                    if type(ins).__name__ == 'InstMatmult' and \
                            cum not in targets:
                        stripped += 1
                    else:
                        keep_cum.append(cum)
                        new_ups.append(u)
                if len(new_ups) != len(ups):
                    si.on_update = new_ups
                    ins.sync_info = si
        import bisect
        for bb in blocks:
            for ins in bb.instructions:
                si = ins.sync_info
                if si is None or not si.on_wait:
                    continue
                ch = False
                ws = list(si.on_wait)
                for w in ws:
                    if w.id == sem and w.wait_mode == 'sem-ge-imm':
                        nv = bisect.bisect_right(keep_cum, w.wait_value)
                        if nv != w.wait_value:
                            w.wait_value = nv
                            ch = True
                if ch:
                    si.on_wait = ws
                    ins.sync_info = si
    return stripped


# ---------------------------------------------------------------------------
# device program
# ---------------------------------------------------------------------------
def build_nc(t_steps=T, n_cores=N_CORES):
    TS = t_steps
    TB = BL * TS           # (t, b) columns per LSTM core
    BT = BC * TS           # (b, t) columns per CRF core (b-major)
    NPAIR = TS + 1         # transition pairs incl. START->t0 and tlast->END
    N_EV = (TS - 1) // 8   # renorm events

    nc = bass.Bass("TRN2", target_bir_lowering=False, debug=False,
                   num_devices=n_cores)

    # inputs (all staged per-core on host)
    xT = nc.dram_tensor("xT", [2, 128, TB], BF16, kind="ExternalInput")
    wihT = nc.dram_tensor("wihT", [2, 128, 4 * H], BF16, kind="ExternalInput")
    whhT = nc.dram_tensor("whhT", [4, 128, 4 * H], BF16, kind="ExternalInput")
    biasT = nc.dram_tensor("biasT", [128, 16], F32, kind="ExternalInput")
    woutT = nc.dram_tensor("woutT", [4, 128, K], BF16, kind="ExternalInput")
    bout = nc.dram_tensor("bout", [K, 1], F32, kind="ExternalInput")
    trans = nc.dram_tensor("trans", [K, K], F32, kind="ExternalInput")
    transT = nc.dram_tensor("transT", [K, K], F32, kind="ExternalInput")
    dirsel = nc.dram_tensor("dirsel", [K, 2], F32, kind="ExternalInput")
    tags_ext = nc.dram_tensor("tags_ext", [BC, TS + 2], F32, kind="ExternalInput")
    tags_flat = nc.dram_tensor("tags_flat", [1, BT], F32, kind="ExternalInput")
    iota_row = nc.dram_tensor("iota_row", [128, K], F32, kind="ExternalInput")
    iota_kp = nc.dram_tensor("iota_kp", [K, 1], F32, kind="ExternalInput")
    ident = nc.dram_tensor("ident", [128, 128], BF16, kind="ExternalInput")
    out = nc.dram_tensor("out", [1, BC], F32, kind="ExternalOutput")

    # collective bounce buffers
    cc_in = nc.dram_tensor("cc_in", [2 * K, BT], F32)
    cc_out = nc.dram_tensor("cc_out", [K, BT], F32)

    with tile.TileContext(nc) as tc:
        _body(tc, locals(), TS, TB, BT, NPAIR, N_EV)
    _strip_mm_incs(nc)
    return nc


def _body(tc, io, TS, TB, BT, NPAIR, N_EV):
    from contextlib import ExitStack
    nc = tc.nc
    xT, wihT, whhT, biasT, woutT = io['xT'], io['wihT'], io['whhT'], io['biasT'], io['woutT']
    bout, trans, transT, dirsel = io['bout'], io['trans'], io['transT'], io['dirsel']
    tags_ext, tags_flat, iota_row, iota_kp = io['tags_ext'], io['tags_flat'], io['iota_row'], io['iota_kp']
    ident = io['ident']
    out, cc_in, cc_out = io['out'], io['cc_in'], io['cc_out']

    with ExitStack() as top:
        persist = top.enter_context(tc.tile_pool(name="persist", bufs=1))

        # persistent tiles
        em_sb = persist.tile([K, TB], F32)           # partial emissions (t,b)
        bias_sb = persist.tile([128, 16], F32)
        nc.sync.dma_start(bias_sb[:], biasT[:, :])
        trans_sb = persist.tile([K, K], F32)
        nc.sync.dma_start(trans_sb[:], trans[:, :])
        transT_sb = persist.tile([K, K], F32)
        nc.sync.dma_start(transT_sb[:], transT[:, :])
        dirsel_sb = persist.tile([K, 2], F32)
        nc.sync.dma_start(dirsel_sb[:], dirsel[:, :])
        bout_sb = persist.tile([K, 1], F32)
        nc.sync.dma_start(bout_sb[:], bout[:, :])
        iota_row_sb = persist.tile([128, K], F32)
        nc.sync.dma_start(iota_row_sb[:], iota_row[:, :])
        iota_kp_sb = persist.tile([K, 1], F32)
        nc.sync.dma_start(iota_kp_sb[:], iota_kp[:, :])
        tagsflat_sb = persist.tile([1, BT], F32)
        nc.sync.dma_start(tagsflat_sb[:], tags_flat[:, :])
        ones32 = persist.tile([K, 1], F32)
        nc.vector.memset(ones32[:], 1.0)
        ones1x32 = persist.tile([1, K], F32)
        nc.vector.memset(ones1x32[:], 1.0)
        ident_sb = persist.tile([128, 128], BF16)
        nc.sync.dma_start(ident_sb[:], ident[:, :])

        # ---------------- phase BC pool (xg + recurrence state) -------------
        with ExitStack() as bc_stack:
            bcpool = bc_stack.enter_context(tc.tile_pool(name="bcpool", bufs=1))
            xg_sb = bcpool.tile([128, 16 * TB], BF16)

            # ---------------- phase B: Xg = W_ih_p @ X^T + bias -------------
            if True:
                bpool = bc_stack.enter_context(tc.tile_pool(name="bpool", bufs=1))
                bxpool = bc_stack.enter_context(tc.tile_pool(name="bxpool", bufs=2))
                bpsum = bc_stack.enter_context(
                    tc.tile_pool(name="bpsum", bufs=2, space="PSUM"))
                wi0 = bpool.tile([128, 4 * H], BF16)
                nc.sync.dma_start(wi0[:], wihT[0, :, :])
                wi1 = bpool.tile([128, 4 * H], BF16)
                nc.sync.dma_start(wi1[:], wihT[1, :, :])
                NCH = min(512, TB)
                NN = TB // NCH
                xs_cur = {}

                def load_x_chunk(n):
                    xs0 = bxpool.tile([128, NCH], BF16, tag="xs0")
                    nc.sync.dma_start(xs0[:], xT[0, :, n * NCH:(n + 1) * NCH])
                    xs1 = bxpool.tile([128, NCH], BF16, tag="xs1")
                    nc.sync.dma_start(xs1[:], xT[1, :, n * NCH:(n + 1) * NCH])
                    xs_cur[0], xs_cur[1] = xs0, xs1

                def emit_xg_unit(j, n):
                    if j == 0:
                        load_x_chunk(n)
                    ps = bpsum.tile([128, NCH], F32, tag="bps")
                    nc.tensor.matmul(ps[:], wi0[:, j * 128:(j + 1) * 128],
                                     xs_cur[0][:], start=True, stop=False)
                    nc.tensor.matmul(ps[:], wi1[:, j * 128:(j + 1) * 128],
                                     xs_cur[1][:], start=False, stop=True)
                    dst = xg_sb[:, j * TB + n * NCH: j * TB + (n + 1) * NCH]
                    if (j + n) % 2 == 0:
                        nc.scalar.activation(dst, ps[:], AF.Identity,
                                             bias=bias_sb[:, j:j + 1])
                    else:
                        nc.vector.tensor_scalar_add(dst, ps[:],
                                                    bias_sb[:, j:j + 1])

                # prologue: first t-chunk (n=0) of Xg for every j; the rest is
                # interleaved into the recurrence to fill PE stalls
                xg_work = []
                for n in range(NN):
                    for j in range(16):
                        if n == 0:
                            emit_xg_unit(j, n)
                        else:
                            xg_work.append((j, n))

            # ---------------- phase C: recurrence + fused emissions ---------
            with ExitStack() as c_stack:
                cpool = c_stack.enter_context(tc.tile_pool(name="cpool", bufs=1))
                whh_sb = cpool.tile([128, 4 * 4 * H], BF16)
                for c in range(4):
                    nc.sync.dma_start(
                        whh_sb[:, c * 4 * H:(c + 1) * 4 * H], whhT[c, :, :])
                wout_sb = cpool.tile([128, 4 * K], BF16)
                for c in range(4):
                    nc.sync.dma_start(wout_sb[:, c * K:(c + 1) * K],
                                      woutT[c, :, :])

                spool = c_stack.enter_context(tc.tile_pool(name="spool", bufs=2))
                qpool = c_stack.enter_context(tc.tile_pool(name="qpool", bufs=3))
                gpsum = c_stack.enter_context(
                    tc.tile_pool(name="gpsum", bufs=2, space="PSUM"))
                empsum = c_stack.enter_context(
                    tc.tile_pool(name="empsum", bufs=2, space="PSUM"))

                # single stream, half-split h/c; Xg preloaded into PSUM via
                # identity matmul so gates = PSUM directly (no DVE add)
                h_prev = spool.tile([128, 64], BF16, tag="h")
                nc.vector.memset(h_prev[:], 0.0)
                c_prev = spool.tile([128, 64], F32, tag="c")
                nc.vector.memset(c_prev[:], 0.0)

                xg_v = xg_sb[:].rearrange("p (j t b) -> p j t b", j=16, t=TS)
                em_copies = []
                for t in range(TS):
                    gps = gpsum.tile([128, 256], F32, tag=f"g{t % 2}",
                                     name="gps")
                    for gt in range(2):
                        nc.tensor.matmul(
                            gps[:, gt * 128:(gt + 1) * 128], ident_sb[:],
                            xg_v[:, 8 * gt:8 * gt + 8, t, :],
                            start=(gt == 0), stop=False)
                    for c_in in range(4):
                        for j in range(16):
                            nc.tensor.matmul(
                                gps[:, j * 16:(j + 1) * 16],
                                whh_sb[:, c_in * 4 * H + j * 128:
                                       c_in * 4 * H + (j + 1) * 128],
                                h_prev[:, c_in * 16:(c_in + 1) * 16],
                                start=False,
                                stop=(c_in == 3 and j == 15))
                    # stall fillers: previous step's emissions + deferred Xg
                    if t > 0:
                        em_ps = empsum.tile([K, BL], F32)
                        for c in range(4):
                            nc.tensor.matmul(
                                em_ps[:], wout_sb[:, c * K:(c + 1) * K],
                                h_prev[:, c * 16:(c + 1) * 16],
                                start=(c == 0), stop=(c == 3))
                        em_copies.append((t - 1, em_ps))
                    if t % 2 == 0 and xg_work:
                        emit_xg_unit(*xg_work.pop(0))

                    hn = qpool.tile([128, 64], BF16, tag="hn")
                    cn = qpool.tile([128, 64], F32, tag="cn")
                    sig = qpool.tile([128, 192], F32, tag="sig")
                    nc.scalar.activation(sig[:], gps[:, 0:192], AF.Sigmoid)
                    nc.vector.tensor_mul(cn[:], sig[:, 64:128], c_prev[:])
                    tg = qpool.tile([128, 64], F32, tag="tg")
                    nc.scalar.activation(tg[:], gps[:, 192:256], AF.Tanh)
                    tmp = qpool.tile([128, 64], F32, tag="tmp")
                    nc.vector.tensor_mul(tmp[:], sig[:, 0:64], tg[:])
                    nc.vector.tensor_add(cn[:], cn[:], tmp[:])
                    tc_sb = qpool.tile([128, 64], F32, tag="tc")
                    nc.scalar.activation(tc_sb[:], cn[:], AF.Tanh)
                    nc.vector.tensor_mul(hn[:], sig[:, 128:192], tc_sb[:])
                    h_prev, c_prev = hn, cn
                    if em_copies:
                        te, eps = em_copies.pop()
                        nc.vector.tensor_copy(
                            em_sb[:, te * BL:(te + 1) * BL], eps[:])
                while xg_work:
                    emit_xg_unit(*xg_work.pop(0))
                em_ps = empsum.tile([K, BL], F32)
                for c in range(4):
                    nc.tensor.matmul(em_ps[:], wout_sb[:, c * K:(c + 1) * K],
                                     h_prev[:, c * 16:(c + 1) * 16],
                                     start=(c == 0), stop=(c == 3))
                nc.vector.tensor_copy(
                    em_sb[:, (TS - 1) * BL:TS * BL], em_ps[:])

        # ---------------- phase D: exchange + finalize emissions ------------
        with ExitStack() as d_stack:
            dpool = d_stack.enter_context(tc.tile_pool(name="dpool", bufs=1))
            # combine normal / time-reversed view by direction selector,
            # writing the result in b-major layout (col = bl*TS + t) so the
            # collective DMAs are contiguous
            cc_pre = dpool.tile([K, TB], F32)
            em_v = em_sb[:].rearrange("p (t b) -> p t b", t=TS)
            em_rv = em_v[:, ::-1, :]
            tmp_r = dpool.tile([K, TB], F32)
            tmp_r_bm = tmp_r[:].rearrange("p (b t) -> p t b", b=BL)
            cc_pre_bm = cc_pre[:].rearrange("p (b t) -> p t b", b=BL)
            nc.vector.tensor_scalar_mul(tmp_r_bm, em_rv, dirsel_sb[:, 1:2])
            nc.vector.scalar_tensor_tensor(
                cc_pre_bm, em_v, dirsel_sb[:, 0:1], tmp_r_bm,
                ALU.mult, ALU.add)
            for h in range(2):
                nc.sync.dma_start(
                    cc_in.ap()[32 * h:32 * h + 32, :],
                    cc_pre[:, 8 * h * TS:(8 * h + 8) * TS])
            nc.gpsimd.collective_compute(
                "ReduceScatter", ALU.add,
                ins=[cc_in.ap()], outs=[cc_out.ap()],
                replica_groups=[[0, 1], [2, 3], [4, 5], [6, 7]])
            em_fin = persist.tile([K, BT], F32)
            rs_sb = dpool.tile([K, BT], F32)
            nc.sync.dma_start(rs_sb[:], cc_out[:, :])
            nc.scalar.activation(em_fin[:], rs_sb[:], AF.Identity,
                                 bias=bout_sb[:, 0:1])
        expE = persist.tile([K, BT], F32)
        nc.scalar.activation(expE[:], em_fin[:], AF.Exp)

        # small tiles that cross the E/F phase boundary
        e_tot = persist.tile([1, BC], F32)
        t_tot = persist.tile([1, BC], F32)
        expT_sb = persist.tile([K, K], F32)
        expTs = persist.tile([K, 1], F32)
        expTe = persist.tile([K, 1], F32)
        k_acc = persist.tile([1, BC], I32)

        # ---------------- phase E: gold-path scores -------------------------
        with ExitStack() as e_stack:
            epool = e_stack.enter_context(tc.tile_pool(name="epool", bufs=2))
            epsum = e_stack.enter_context(
                tc.tile_pool(name="epsum", bufs=2, space="PSUM"))
            cpsum = e_stack.enter_context(
                tc.tile_pool(name="cpsum", bufs=1, space="PSUM"))

            # transition counts over extended sequences
            C_ps = cpsum.tile([K, BC * K], F32)
            chunk_starts = list(range(0, NPAIR, 128))
            for b in range(BC):
                for ci, s0 in enumerate(chunk_starts):
                    sz = min(128, NPAIR - s0)
                    tp = epool.tile([128, 1], F32, tag="tp")
                    nc.sync.dma_start(tp[:sz, :],
                                      tags_ext[b:b + 1, s0:s0 + sz])
                    tn = epool.tile([128, 1], F32, tag="tn")
                    nc.sync.dma_start(tn[:sz, :],
                                      tags_ext[b:b + 1, s0 + 1:s0 + 1 + sz])
                    ohp = epool.tile([128, K], F32, tag="ohp")
                    nc.vector.tensor_scalar(ohp[:sz, :], iota_row_sb[:sz, :],
                                            tp[:sz, :], None, ALU.is_equal)
                    ohn = epool.tile([128, K], F32, tag="ohn")
                    nc.vector.tensor_scalar(ohn[:sz, :], iota_row_sb[:sz, :],
                                            tn[:sz, :], None, ALU.is_equal)
                    nc.tensor.matmul(C_ps[:, b * K:(b + 1) * K],
                                     ohp[:sz, :], ohn[:sz, :],
                                     start=(ci == 0),
                                     stop=(ci == len(chunk_starts) - 1))
            trans8 = epool.tile([K, BC * K], F32, tag="trans8")
            for b in range(BC):
                nc.vector.tensor_copy(trans8[:, b * K:(b + 1) * K], trans_sb[:])
            tcmul = epool.tile([K, BC * K], F32, tag="tcmul")
            nc.vector.tensor_mul(tcmul[:], C_ps[:], trans8[:])
            tred = epool.tile([K, BC], F32, tag="tred")
            nc.vector.tensor_reduce(
                tred[:], tcmul[:].rearrange("p (b k) -> p b k", b=BC),
                mybir.AxisListType.X, ALU.add)
            ttot_ps = cpsum.tile([1, BC], F32, tag="ttot")
            nc.tensor.matmul(ttot_ps[:], ones32[:], tred[:],
                             start=True, stop=True)
            nc.vector.tensor_copy(t_tot[:], ttot_ps[:])

            # emission scores: one-hot mask + partition sum + t-reduction
            NSL = min(512, BT)
            for s in range(BT // NSL):
                sl = slice(s * NSL, (s + 1) * NSL)
                tb_ps = epsum.tile([K, NSL], F32, tag="tbps")
                nc.tensor.matmul(tb_ps[:], ones1x32[:], tagsflat_sb[:, sl],
                                 start=True, stop=True)
                ohm = epool.tile([K, NSL], F32, tag="ohm")
                nc.vector.tensor_scalar(ohm[:], tb_ps[:], iota_kp_sb[:],
                                        None, ALU.is_equal)
                nc.vector.tensor_mul(ohm[:], ohm[:], em_fin[:, sl])
                es_ps = epsum.tile([1, NSL], F32, tag="esps")
                nc.tensor.matmul(es_ps[:], ones32[:], ohm[:],
                                 start=True, stop=True)
                nb = NSL // TS
                nc.vector.tensor_reduce(
                    e_tot[:, s * nb:(s + 1) * nb],
                    es_ps[:].rearrange("p (b t) -> p b t", t=TS),
                    mybir.AxisListType.X, ALU.add)

        # ------------- phase F: CRF forward recurrence ------------------
        with ExitStack() as f_stack:
            fpool = f_stack.enter_context(tc.tile_pool(name="fpool", bufs=2))
            fpsum = f_stack.enter_context(
                tc.tile_pool(name="fpsum", bufs=2, space="PSUM"))

            nc.scalar.activation(expT_sb[:], trans_sb[:], AF.Exp)
            nc.scalar.activation(expTs[:], transT_sb[:, START:START + 1],
                                 AF.Exp)
            nc.scalar.activation(expTe[:], trans_sb[:, END:END + 1], AF.Exp)

            expE_v = expE[:].rearrange("p (b t) -> p b t", b=BC)
            a_cur = fpool.tile([K, BC], F32, tag="a")
            nc.vector.tensor_scalar_mul(a_cur[:], expE_v[:, :, 0], expTs[:])
            nc.vector.memset(k_acc[:], 0)

            for t in range(1, TS):
                a_ps = fpsum.tile([K, BC], F32, tag="aps")
                nc.tensor.matmul(a_ps[:], expT_sb[:], a_cur[:],
                                 start=True, stop=True)
                a_nxt = fpool.tile([K, BC], F32, tag="a")
                nc.vector.tensor_mul(a_nxt[:], a_ps[:], expE_v[:, :, t])
                a_cur = a_nxt
                if t % 8 == 0:
                    zps_t = fpsum.tile([K, BC], F32, tag="fps")
                    z_ps = zps_t[0:1, :]
                    nc.tensor.matmul(z_ps[:], ones32[:], a_cur[:],
                                     start=True, stop=True)
                    z_sb = fpool.tile([1, BC], F32, tag="zsb")
                    nc.vector.tensor_copy(z_sb[:], z_ps[:])
                    e_i = fpool.tile([1, BC], I32, tag="ei")
                    nc.vector.tensor_scalar(e_i[:], z_sb[:].bitcast(I32),
                                            23, None,
                                            ALU.logical_shift_right)
                    nc.vector.tensor_add(k_acc[:], k_acc[:], e_i[:])
                    sc_i = fpool.tile([1, BC], I32, tag="sci")
                    nc.vector.tensor_scalar(sc_i[:], e_i[:], -1, 254,
                                            ALU.mult, ALU.add)
                    nc.vector.tensor_scalar(sc_i[:], sc_i[:], 23, None,
                                            ALU.logical_shift_left)
                    bc_ps = fpsum.tile([K, BC], F32, tag="fps")
                    nc.tensor.matmul(bc_ps[:], ones1x32[:],
                                     sc_i[:].bitcast(F32),
                                     start=True, stop=True)
                    a_sc = fpool.tile([K, BC], F32, tag="a")
                    nc.vector.tensor_mul(a_sc[:], a_cur[:], bc_ps[:])
                    a_cur = a_sc

            zf_t = fpsum.tile([K, BC], F32, tag="fps")
            zf_ps = zf_t[0:1, :]
            nc.tensor.matmul(zf_ps[:], expTe[:], a_cur[:],
                             start=True, stop=True)
            logz = fpool.tile([1, BC], F32, tag="logz")
            nc.scalar.activation(logz[:], zf_ps[:], AF.Ln)
            k_f = fpool.tile([1, BC], F32, tag="kf")
            nc.vector.tensor_copy(k_f[:], k_acc[:])
            # nll = logz + ln2*(sum e) - 127*ln2*n_ev - e_tot - t_tot
            nll = fpool.tile([1, BC], F32, tag="nll")
            nc.vector.tensor_scalar(nll[:], k_f[:], LN2,
                                    -127.0 * LN2 * N_EV, ALU.mult, ALU.add)
            nc.vector.tensor_add(nll[:], nll[:], logz[:])
            nc.vector.tensor_sub(nll[:], nll[:], e_tot[:])
            nc.vector.tensor_sub(nll[:], nll[:], t_tot[:])
            nc.sync.dma_start(out[:, :], nll[:])


# ---------------------------------------------------------------------------
# host side
# ---------------------------------------------------------------------------
def _perm_rows(W):
    # gate-major blocks reordered i,f,o,g (pytorch order is i,f,g,o)
    out = np.empty_like(W)
    out[0:1024] = W[0:1024]          # i, f
    out[1024:1536] = W[1536:2048]    # o
    out[1536:2048] = W[1024:1536]    # g
    return out


def make_in_maps(inputs, t_steps=T):
    TS = t_steps
    X = np.asarray(inputs['X'], np.float32)
    tags = np.asarray(inputs['tags']).astype(np.int64)
    W = {d: (np.asarray(inputs[f'W_ih_{d}'], np.float32),
             np.asarray(inputs[f'W_hh_{d}'], np.float32),
             np.asarray(inputs[f'b_ih_{d}'], np.float32)
             + np.asarray(inputs[f'b_hh_{d}'], np.float32))
         for d in ('f', 'b')}
    W_out = np.asarray(inputs['W_out'], np.float32)
    b_out = np.asarray(inputs['b_out'], np.float32)
    trans = np.asarray(inputs['transitions'], np.float32)

    iota_row = np.tile(np.arange(K, dtype=np.float32), (128, 1))
    iota_kp = np.arange(K, dtype=np.float32)[:, None]

    maps = []
    for c in range(N_CORES):
        d = 'f' if c % 2 == 0 else 'b'
        w = c // 2
        b0 = BL * w
        Wih, Whh, bsum = W[d]
        wihT = _perm_rows(Wih).T.astype(ml_dtypes.bfloat16)      # [E, 4H]
        whhT = _perm_rows(Whh).T.astype(ml_dtypes.bfloat16)      # [H, 4H]
        biasT = _perm_rows(bsum[:, None])[:, 0].reshape(16, 128).T.copy()
        wo = W_out[(0 if d == 'f' else H):(H if d == 'f' else 2 * H), :]
        Xs = X[b0:b0 + BL, :TS, :]                               # [BL, TS, E]
        XT = Xs.transpose(2, 1, 0)                               # [E, TS, BL]
        if d == 'b':
            XT = XT[:, ::-1, :]
        crf = tags[b0 + (0 if d == 'f' else BC):
                   b0 + (BC if d == 'f' else 2 * BC), :TS]
        text = np.concatenate(
            [np.full((BC, 1), START), crf, np.full((BC, 1), END)],
            1).astype(np.float32)
        maps.append({
            "xT": np.ascontiguousarray(
                XT.reshape(2, 128, TS * BL)).astype(ml_dtypes.bfloat16),
            "wihT": np.ascontiguousarray(wihT.reshape(2, 128, 4 * H)),
            "whhT": np.ascontiguousarray(whhT.reshape(4, 128, 4 * H)),
            "biasT": np.ascontiguousarray(biasT).astype(np.float32),
            "woutT": np.ascontiguousarray(
                wo.reshape(4, 128, K)).astype(ml_dtypes.bfloat16),
            "bout": b_out[:, None].astype(np.float32),
            "trans": trans,
            "transT": np.ascontiguousarray(trans.T),
            "dirsel": np.tile(np.float32([1.0, 0.0] if d == 'f' else [0.0, 1.0]),
                              (K, 1)).astype(np.float32),
            "tags_ext": text,
            "tags_flat": crf.reshape(1, -1).astype(np.float32),
            "iota_row": iota_row,
            "iota_kp": iota_kp,
            "ident": np.eye(128, dtype=ml_dtypes.bfloat16),
        })
    return maps


def assemble_out(results):
    nll = np.zeros(B, np.float32)
    for c in range(N_CORES):
        w = c // 2
        off = 16 * w + (0 if c % 2 == 0 else BC)
        nll[off:off + BC] = results[c]["out"][0]
    return nll


_CACHED = {}


def kernel(**inputs):
    masks = np.asarray(inputs['masks'], np.float32)
    assert np.all(masks == 1.0), "kernel assumes masks == 1 (setup_inputs)"
    if 'nc' not in _CACHED:
        nc = build_nc()
        _split_multiwait(nc)
        _CACHED['nc'] = nc
    in_maps = make_in_maps(inputs)
    res = run_bass_kernel_spmd(_CACHED['nc'], in_maps,
                               core_ids=list(range(N_CORES)))
    return assemble_out(res.results)
